# revision 7
# baseline (speedup 1.0000x reference)
"""Trainium2 Bass kernel for nn_Attention (B=2, N=4096, D=1024, 16 heads).

Sharding: 8 cores = 2 (batch) x 4 (head groups of 4 heads, Megatron TP).
Each core computes qkv for its 4 heads, flash-style attention (S^T layout,
softmax denominator via a ones-column folded into the V stationary), and its
partial output projection. The 4 partial projections per batch are summed on
the host during unshard (the TP all-reduce), plus the bias.

v3: the QKV prologue is folded into the first attention chunk. The ACT
(scalar) engine is the hard bottleneck (512 exp instructions, ~1.0us each,
~510us floor); any PE work that is not covered by ACT time is pure span.
The backbone starts as soon as K/Q for the first 1024 keys/queries exist;
V tiles and the remaining K/Q chunks are produced inside the c0 j-loops,
interleaved behind the QK/AV matmuls. Norm / projection / leftover QKV run
as background micro-steps in later chunks, as in v2.
"""

from collections import deque

import numpy as np

import concourse.bacc as bacc
import concourse.mybir as mybir
import concourse.tile as tile

B = 2
N = 4096
D = 1024
HL = 4          # heads per core
HD = 64         # head dim
DG = HL * HD    # 256 = per-core d' width
SCALE = HD ** -0.5

FP32 = mybir.dt.float32
BF16 = mybir.dt.bfloat16
I16 = mybir.dt.int16
MULT = mybir.AluOpType.mult
ADD = mybir.AluOpType.add
EXP = mybir.ActivationFunctionType.Exp

# ---- custom DVE op: Schraudolph-bf16 exp correction factor ----
# e1 = bitcast_bf16(int16(S*EXP_A + EXP_B)) = 2^floor(t)*(1+frac(t)),
# t = S*SCALE*log2(e).  G = 2^frac/(1+frac) approximated (even in
# h = t-round(t)) by 1 + |h|*(C2 + C3*|h|); et = G * e1.
EXP_C0 = 0.18033688011112042       # SCALE * log2(e)
EXP_C1 = 12582912.0                # 1.5 * 2^23 round-to-int magic
EXP_C2 = -0.22853454042930804
EXP_C3 = 0.23712897645623385
EXP_A = 23.083120654223414         # 128 * SCALE * log2(e)
EXP_B = 16256.0                    # 127 * 128


def _make_exp_corr():
    from concourse import dve_ops
    from concourse.dve_spec import (Spec, Src0, C0, C1, C2, C3, One, AluOp,
                                    Bin, lower, _spill_c3_to_src1)
    from concourse.dve_uop import DveOpSpec

    for op in dve_ops.OPS:
        if op.name == "EXP_CORR_ANT":
            return op
    name = "EXP_CORR_ANT"
    t = Src0 * C0
    v = t + C1
    w = v - C1
    a = Bin(AluOp.ABSOLUTE_DIFF, t, w)
    body = _spill_c3_to_src1(One + a * (C2 + a * C3))

    def _ref(in0, in1, s0, s1, imm2):
        tt = (in0 * np.float32(s0)).astype(np.float32)
        ww = ((tt + np.float32(s1)).astype(np.float32)
              - np.float32(s1)).astype(np.float32)
        aa = np.abs((tt - ww).astype(np.float32))
        return (1.0 + aa * (np.float32(imm2) + aa * in1)).astype(np.float32)

    spec = Spec(body=body, reference=_ref)
    row = dve_ops._CUSTOM_DVE_ROW_BASE + len(dve_ops.OPS)
    assert row < 0x20
    shas = {}
    for ver in ("v3", "v4"):
        uops = lower(spec, ver=ver)
        shas[ver] = DveOpSpec(name=name, opcode=row, uops=uops,
                              rd1_en=True).sha(ver)
    op = dve_ops.DveOp(name, spec, subdim=False, uops_sha=shas)
    dve_ops.OPS.append(op)
    dve_ops.CUSTOM_DVE_SPECS[name] = spec
    dve_ops._SUB_OPCODE_FOR_NAME[name] = row
    return op


EXP_CORR = _make_exp_corr()


def _build(n=N):
    nc = bacc.Bacc("TRN2", target_bir_lowering=False, debug=False)

    xT = nc.declare_dram_parameter("xT", [D, n], BF16, isOutput=False)
    wqT = nc.declare_dram_parameter("wqT", [D, DG], BF16, isOutput=False)
    wkT = nc.declare_dram_parameter("wkT", [D, DG], BF16, isOutput=False)
    wvT = nc.declare_dram_parameter("wvT", [D, DG], BF16, isOutput=False)
    wpT2 = nc.declare_dram_parameter("wpT2", [128, 2, D], BF16, isOutput=False)
    out = nc.declare_dram_parameter("out", [n, D], BF16, isOutput=True)

    DT = D // 128        # 8 contraction tiles for qkv
    NT = n // 128        # key tiles
    QC = min(1024, n)    # qkv prefix group width
    NQC = n // QC
    NC = n // 512        # attention i-chunks

    xT_r = xT.rearrange("(dt p) n -> dt p n", p=128)

    with tile.TileContext(nc) as tc:
        with (
            tc.tile_pool(name="sb", bufs=1) as sb,
            tc.tile_pool(name="wkp", bufs=1) as wkpool,
            tc.tile_pool(name="ps", bufs=1, space="PSUM") as ps,
        ):
            # ---- persistent SBUF tiles ----
            xt = sb.tile([128, DT, n], BF16, tag="xt")
            wq_t = sb.tile([128, DT, DG], BF16, tag="wq")
            wk_t = sb.tile([128, DT, DG], BF16, tag="wk")
            wv_t = sb.tile([128, DT, DG], BF16, tag="wv")
            wp_t = sb.tile([128, 2, D], BF16, tag="wp")
            qt = sb.tile([128, 2, n], BF16, tag="qt")
            kt = sb.tile([128, 2, n], BF16, tag="kt")
            # V stationary, heads at stride 65 (64 V dims + ones col);
            # head h's AV weight window is [65h : 65h+128] (the tail 63
            # cols are the next head's V / zero pad): a full 128-col
            # weight enables the FWL fast path, out rows 65:127 are
            # garbage and never read, the denominator stays at row 64.
            vaug = sb.tile([128, NT, 325], BF16, tag="vaug")
            otn = sb.tile([128, 2, n], BF16, tag="otn")

            # ---- DMA emission: first-needed first, batched per tensor ----
            wqT_p = wqT.rearrange("(dt p) m -> p dt m", p=128)
            wkT_p = wkT.rearrange("(dt p) m -> p dt m", p=128)
            wvT_p = wvT.rearrange("(dt p) m -> p dt m", p=128)
            xT_p = xT.rearrange("(dt p) n -> p dt n", p=128)
            # load the exp table-set (~2.7us) while the DMAs stream
            scr = wkpool.tile([1, 8], FP32, tag="scr", bufs=1, name="scr")
            nc.vector.memset(scr[:, :], 0.0)
            nc.scalar.activation(scr[:, :], scr[:, :], EXP)
            onesb = wkpool.tile([1, 64], BF16, tag="onesb", bufs=1,
                                name="onesb")
            nc.vector.memset(onesb[:, :], 1.0)
            c3t = wkpool.tile([128, 1], FP32, tag="c3t", bufs=1, name="c3t")
            nc.vector.memset(c3t[:, :], EXP_C3)
            # wk and the first x chunk in half-batches: the pre-head K
            # matmuls for dt 0-3 start as soon as the first halves land
            nc.sync.dma_start(wk_t[:, 0:4, :], wkT_p[:, 0:4, :])
            nc.sync.dma_start(xt[:, 0:4, 0:512], xT_p[:, 0:4, 0:512])
            nc.sync.dma_start(wk_t[:, 4:8, :], wkT_p[:, 4:8, :])
            nc.sync.dma_start(xt[:, 4:8, 0:512], xT_p[:, 4:8, 0:512])
            nc.sync.dma_start(wq_t[:, 0:4, :], wqT_p[:, 0:4, :])
            nc.sync.dma_start(wq_t[:, 4:8, :], wqT_p[:, 4:8, :])
            nc.sync.dma_start(wv_t[:, 0:4, :], wvT_p[:, 0:4, :])
            nc.sync.dma_start(xt[:, :, 512:1024], xT_p[:, :, 512:1024])
            nc.sync.dma_start(wv_t[:, 4:8, :], wvT_p[:, 4:8, :])
            nc.sync.dma_start(xt[:, :, 1024:2048], xT_p[:, :, 1024:2048])
            nc.sync.dma_start(wp_t[:, :, :], wpT2[:, :, :])
            nc.sync.dma_start(xt[:, :, 2048:3072], xT_p[:, :, 2048:3072])
            nc.sync.dma_start(xt[:, :, 3072:4096], xT_p[:, :, 3072:4096])
            nc.vector.memset(vaug[:, :, 260:325], 0.0)
            for j in range(NT):
                vj = vaug[:, j, 0:260].rearrange("p (h s) -> p h s", s=65)
                nc.vector.memset(vj[:, :, 64], 1.0)

            # ---- background micro-step machinery ----
            bg = deque()

            def drain(k):
                done = 0
                while bg and done < k:
                    try:
                        next(bg[0])
                        done += 1
                    except StopIteration:
                        bg.popleft()

            # ---- QKV building blocks ----
            def qk_group_bg(w_t, dst, m, c5):
                # [128, 512] background group on the aux tag
                kp = ps.tile([128, 512], FP32, tag="aux", bufs=2, name="qkb")
                for dt_i in range(DT):
                    nc.tensor.matmul(
                        kp[:, :],
                        w_t[:, dt_i, m * 128:(m + 1) * 128],
                        xt[:, dt_i, c5 * 512:(c5 + 1) * 512],
                        start=(dt_i == 0), stop=(dt_i == DT - 1),
                    )
                    yield
                nc.vector.tensor_copy(dst[:, m, c5 * 512:(c5 + 1) * 512],
                                      kp[:, :])
                yield

            def v_gen(j):
                vp = ps.tile([128, DG], FP32, tag="aux", bufs=2, name="vp")
                for dt_i in range(DT):
                    nc.tensor.matmul(
                        vp[:, :],
                        xt[:, dt_i, j * 128:(j + 1) * 128],
                        wv_t[:, dt_i, :],
                        start=(dt_i == 0), stop=(dt_i == DT - 1),
                    )
                    yield
                nc.vector.tensor_copy(
                    vaug[:, j, 0:260].rearrange(
                        "p (h s) -> p h s", s=65)[:, :, 0:64],
                    vp[:, :].rearrange("p (h d) -> p h d", d=64))
                yield

            # ---- normalize + projection generators ----
            def norm_rest(osb, zrow, hh, c):
                pt, odd = hh // 2, hh % 2
                rz = wkpool.tile([1, 512], FP32, tag="rz", bufs=2, name="rz")
                nc.vector.reciprocal_approx_fast(rz[:, :], zrow[:, :])
                yield
                rzs = wkpool.tile([64, 512], FP32, tag="rzs", bufs=4,
                                  name="rzs")
                nc.gpsimd.partition_broadcast(rzs[:, :], rz[:, :])
                yield
                cs = slice(c * 512, (c + 1) * 512)
                if odd == 0:
                    nc.vector.tensor_tensor(otn[0:64, pt, cs],
                                            osb[0:64, :], rzs[:, :], MULT)
                    yield
                else:
                    ohst = wkpool.tile([64, 512], BF16, tag="ohst", bufs=4,
                                       name="ohst")
                    nc.vector.tensor_tensor(ohst[:, :], osb[0:64, :],
                                            rzs[:, :], MULT)
                    yield
                    nc.sync.dma_start(otn[64:128, pt, cs], ohst[:, :])
                    yield

            def norm_last(osb, zrow, hh, c):
                # tail variant: broadcast 1/z with a K=1 PE matmul into a
                # freed ot PSUM bank instead of the serial gpsimd path.
                pt, odd = hh // 2, hh % 2
                rz = wkpool.tile([1, 512], FP32, tag="rz", bufs=2,
                                 name="rz")
                nc.vector.reciprocal_approx_fast(rz[:, :], zrow[:, :])
                rzb = wkpool.tile([1, 512], BF16, tag="rzb", bufs=2,
                                  name="rzb")
                nc.vector.tensor_copy(rzb[:, :], rz[:, :])
                yield
                rzp = ps.tile([128, 512], FP32, tag="ot", bufs=2, name="rzp")
                nc.tensor.matmul(rzp[0:64, :], onesb[0:1, :], rzb[0:1, :],
                                 start=True, stop=True)
                yield
                cs = slice(c * 512, (c + 1) * 512)
                if odd == 0:
                    nc.vector.tensor_tensor(otn[0:64, pt, cs],
                                            osb[0:64, :], rzp[0:64, :], MULT)
                    yield
                else:
                    ohst = wkpool.tile([64, 512], BF16, tag="ohst", bufs=4,
                                       name="ohst")
                    nc.vector.tensor_tensor(ohst[:, :], osb[0:64, :],
                                            rzp[0:64, :], MULT)
                    yield
                    nc.sync.dma_start(otn[64:128, pt, cs], ohst[:, :])
                    yield

            def proj_gen(c):
                for isub in range(4):
                    ib = c * 512 + isub * 128
                    for e in range(2):
                        pj = ps.tile([128, 512], FP32, tag="aux", bufs=2,
                                     name="pj")
                        for pt in range(2):
                            nc.tensor.matmul(
                                pj[:, :],
                                otn[:, pt, ib:ib + 128],
                                wp_t[:, pt, e * 512:(e + 1) * 512],
                                start=(pt == 0), stop=(pt == 1))
                            yield
                        ob = wkpool.tile([128, 512], BF16, tag="ob", bufs=3,
                                         name="ob")
                        nc.vector.tensor_copy(ob[:, :], pj[:, :])
                        nc.sync.dma_start(
                            out[ib:ib + 128, e * 512:(e + 1) * 512],
                            ob[:, :])
                        yield

            # ---- minimal pre-head: K/Q for pair 0, first 512 cols ----
            for _ in qk_group_bg(wk_t, kt, 0, 0):
                pass
            for _ in qk_group_bg(wq_t, qt, 0, 0):
                pass

            # K/Q production queue consumed inside the c0 j-loops.
            # Order = first-needed-first:
            #  c0p0 j>=4 needs K(m0) cols 512:4096; c0p1 needs Q(m1) col
            #  chunk 0 and K(m1) everything; chunk c1 needs Q(*) chunk 1.
            prodq = deque()
            for c5 in range(1, NC):
                prodq.append(qk_group_bg(wk_t, kt, 0, c5))
            prodq.append(qk_group_bg(wq_t, qt, 1, 0))
            prodq.append(qk_group_bg(wk_t, kt, 1, 0))
            for c5 in range(1, NC):
                prodq.append(qk_group_bg(wk_t, kt, 1, c5))

            def drive(q, k):
                done = 0
                while q and done < k:
                    try:
                        next(q[0])
                        done += 1
                    except StopIteration:
                        q.popleft()

            vq = deque(v_gen(j) for j in range(NT))

            # remaining Q as background (512-wide groups); chunk-1 Q
            # first (needed at c1, drained during c0p1)
            bg.append(qk_group_bg(wq_t, qt, 0, 1))
            bg.append(qk_group_bg(wq_t, qt, 1, 1))
            for c5 in range(QC // 512, NC):
                bg.append(qk_group_bg(wq_t, qt, 0, c5))
                bg.append(qk_group_bg(wq_t, qt, 1, c5))

            # ---- attention backbone ----
            for c in range(NC):
                for p in range(2):
                    first_pass = (c == 0 and p == 0)
                    he, ho = 2 * p, 2 * p + 1
                    ot_e = ps.tile([128, 512], FP32, tag="ot", bufs=2,
                                   name="ot_e")
                    ot_o = ps.tile([128, 512], FP32, tag="ot", bufs=2,
                                   name="ot_o")
                    def qk_pair(j):
                        st = ps.tile([128, 1024], FP32, tag="st", bufs=2,
                                     name="st")
                        nc.tensor.matmul(
                            st[:, 0:512],
                            kt[0:64, p, j * 128:(j + 1) * 128],
                            qt[0:64, p, c * 512:(c + 1) * 512],
                            start=True, stop=True)
                        nc.tensor.matmul(
                            st[:, 512:1024],
                            kt[64:128, p, j * 128:(j + 1) * 128],
                            qt[64:128, p, c * 512:(c + 1) * 512],
                            start=True, stop=True)
                        return st

                    pend = None
                    for j in range(NT):
                        st = qk_pair(j)
                        et = sb.tile([128, 1024], BF16, tag="et", bufs=3,
                                     name="et", padded_shape=[128, 2048])
                        if c > 0 and j % 4 == 2:
                            u16 = sb.tile([128, 1024], I16, tag="u16",
                                          bufs=2, name="u16")
                            gb = sb.tile([128, 1024], BF16, tag="gb",
                                         bufs=2, name="gb")
                            nc.vector.tensor_scalar(
                                u16[:, :], st[:, :], EXP_A, EXP_B, MULT, ADD)
                            nc.vector._custom_dve(
                                EXP_CORR, out=gb[:, :], in0=st[:, :],
                                in1=c3t[:, :], s0=EXP_C0, s1=EXP_C1,
                                imm2=EXP_C2)
                            nc.vector.tensor_tensor(
                                et[:, :], gb[:, :],
                                u16[:, :].bitcast(BF16), MULT)
                        else:
                            nc.scalar.activation(et[:, :], st[:, :], EXP,
                                                 scale=SCALE)
                        if pend is not None:
                            pj_, pet = pend
                            nc.tensor.matmul(
                                ot_e[0:128, :], vaug[:, pj_, 65 * he:65 * he + 128],
                                pet[:, 0:512],
                                start=(pj_ == 0), stop=False)
                            nc.tensor.matmul(
                                ot_o[0:128, :], vaug[:, pj_, 65 * ho:65 * ho + 128],
                                pet[:, 512:1024],
                                start=(pj_ == 0), stop=False)
                        pend = (j, et)
                        if first_pass:
                            # produce V[j] (needed by AV next iter) and
                            # push K/Q production along behind the QK/AV.
                            drive(vq, 9)
                            drive(prodq, 4)
                        elif c == 0 and p == 1:
                            drive(prodq, 3)
                            drain(1)
                        else:
                            drain(2 if c >= NC - 2 else 1)
                    pj_, pet = pend
                    nc.tensor.matmul(
                        ot_e[0:128, :], vaug[:, pj_, 65 * he:65 * he + 128],
                        pet[:, 0:512], start=False, stop=True)
                    nc.tensor.matmul(
                        ot_o[0:128, :], vaug[:, pj_, 65 * ho:65 * ho + 128],
                        pet[:, 512:1024], start=False, stop=True)
                    # free the PSUM accumulators with one copy each;
                    # the rest of the normalize chain runs in background
                    last_norms = []
                    pairs = ((he, ot_e), (ho, ot_o))
                    if c == NC - 1 and p == 1:
                        pairs = ((ho, ot_o), (he, ot_e))
                    for hh, ot_h in pairs:
                        osb = wkpool.tile([64, 512], BF16, tag="osb",
                                          bufs=6, name="osb")
                        nc.vector.tensor_copy(osb[:, :], ot_h[0:64, :])
                        zrow = wkpool.tile([1, 512], FP32, tag="zrow",
                                           bufs=4, name="zrow")
                        nc.vector.tensor_copy(zrow[:, :], ot_h[64:65, :])
                        if c == NC - 1 and p == 1:
                            last_norms.append(
                                norm_last(osb, zrow, hh, c))
                        else:
                            bg.append(norm_rest(osb, zrow, hh, c))
                    while last_norms:
                        for ng in list(last_norms):
                            try:
                                next(ng)
                            except StopIteration:
                                last_norms.remove(ng)
                pg = proj_gen(c)
                if c == NC - 1:
                    while bg:
                        drain(64)
                    for _ in pg:
                        pass
                else:
                    bg.append(pg)

            while prodq:
                drive(prodq, 64)
            while vq:
                drive(vq, 64)
            while bg:
                drain(64)

    nc.compile()
    return nc


_CACHED = {}


def _get_nc(n=N):
    if n not in _CACHED:
        _CACHED[n] = _build(n)
    return _CACHED[n]


def _make_in_maps(x, w_qkv, w_proj):
    import ml_dtypes
    bf16 = ml_dtypes.bfloat16
    in_maps = []
    for c in range(8):
        b, g = divmod(c, 4)
        s = slice(g * DG, (g + 1) * DG)
        wp = w_proj[:, s]  # [D(e), 256]
        in_maps.append({
            "xT": np.ascontiguousarray(x[b].T).astype(bf16),
            "wqT": np.ascontiguousarray(w_qkv[0 * D:1 * D][s, :].T).astype(bf16),
            "wkT": np.ascontiguousarray(w_qkv[1 * D:2 * D][s, :].T).astype(bf16),
            "wvT": np.ascontiguousarray(w_qkv[2 * D:3 * D][s, :].T).astype(bf16),
            "wpT2": np.ascontiguousarray(
                wp.T.reshape(2, 128, D).transpose(1, 0, 2)).astype(bf16),
        })
    return in_maps


def kernel(x, w_qkv, w_proj, b_proj):
    from concourse.bass_utils import run_bass_kernel_spmd

    nc = _get_nc(N)
    in_maps = _make_in_maps(np.asarray(x), np.asarray(w_qkv),
                            np.asarray(w_proj))
    res = run_bass_kernel_spmd(nc, in_maps, core_ids=list(range(8)))
    outs = [r["out"].astype(np.float32) for r in res.results]
    full = np.stack([outs[0] + outs[1] + outs[2] + outs[3],
                     outs[4] + outs[5] + outs[6] + outs[7]])
    full += np.asarray(b_proj, dtype=np.float32)[None, None, :]
    return full.astype(np.float32)



# revision 8
# speedup vs baseline: 1.3181x; 1.3181x over previous
"""Trainium2 Bass kernel for nn_Attention (B=2, N=4096, D=1024, 16 heads).

Sharding: 8 cores = 2 (batch) x 4 (head groups of 4 heads, Megatron TP).
Each core computes qkv for its 4 heads, flash-style attention (S^T layout,
softmax denominator via a ones-column folded into the V stationary), and its
partial output projection. The 4 partial projections per batch are summed on
the host during unshard (the TP all-reduce), plus the bias.

v3: the QKV prologue is folded into the first attention chunk. The ACT
(scalar) engine is the hard bottleneck (512 exp instructions, ~1.0us each,
~510us floor); any PE work that is not covered by ACT time is pure span.
The backbone starts as soon as K/Q for the first 1024 keys/queries exist;
V tiles and the remaining K/Q chunks are produced inside the c0 j-loops,
interleaved behind the QK/AV matmuls. Norm / projection / leftover QKV run
as background micro-steps in later chunks, as in v2.
"""

from collections import deque

import numpy as np

import concourse.bacc as bacc
import concourse.mybir as mybir
import concourse.tile as tile

B = 2
N = 4096
D = 1024
HL = 4          # heads per core
HD = 64         # head dim
DG = HL * HD    # 256 = per-core d' width
SCALE = HD ** -0.5

FP32 = mybir.dt.float32
BF16 = mybir.dt.bfloat16
I16 = mybir.dt.int16
MULT = mybir.AluOpType.mult
ADD = mybir.AluOpType.add
EXP = mybir.ActivationFunctionType.Exp

# ---- custom DVE op: Schraudolph-bf16 exp correction factor ----
# e1 = bitcast_bf16(int16(S*EXP_A + EXP_B)) = 2^floor(t)*(1+frac(t)),
# t = S*SCALE*log2(e).  G = 2^frac/(1+frac) approximated (even in
# h = t-round(t)) by 1 + |h|*(C2 + C3*|h|); et = G * e1.
EXP_C0 = 0.0078125                 # 1/128: I2 reads the int16 I1 output
EXP_C1 = 12582912.0                # 1.5 * 2^23 round-to-int magic
EXP_C2 = -0.22853454042930804
EXP_C3 = 0.23712897645623385
EXP_A = 23.083120654223414         # 128 * SCALE * log2(e)
EXP_B = 16256.0                    # 127 * 128


def _make_exp_corr():
    from concourse import dve_ops
    from concourse.dve_spec import (Spec, Src0, C0, C1, C2, C3, One, AluOp,
                                    Bin, lower, _spill_c3_to_src1)
    from concourse.dve_uop import DveOpSpec

    for op in dve_ops.OPS:
        if op.name == "EXP_CORR_ANT":
            return op
    name = "EXP_CORR_ANT"
    t = Src0 * C0
    v = t + C1
    w = v - C1
    a = Bin(AluOp.ABSOLUTE_DIFF, t, w)
    body = _spill_c3_to_src1(One + a * (C2 + a * C3))

    def _ref(in0, in1, s0, s1, imm2):
        tt = (in0 * np.float32(s0)).astype(np.float32)
        ww = ((tt + np.float32(s1)).astype(np.float32)
              - np.float32(s1)).astype(np.float32)
        aa = np.abs((tt - ww).astype(np.float32))
        return (1.0 + aa * (np.float32(imm2) + aa * in1)).astype(np.float32)

    spec = Spec(body=body, reference=_ref)
    row = dve_ops._CUSTOM_DVE_ROW_BASE + len(dve_ops.OPS)
    assert row < 0x20
    shas = {}
    for ver in ("v3", "v4"):
        uops = lower(spec, ver=ver)
        shas[ver] = DveOpSpec(name=name, opcode=row, uops=uops,
                              rd1_en=True).sha(ver)
    op = dve_ops.DveOp(name, spec, subdim=False, uops_sha=shas)
    dve_ops.OPS.append(op)
    dve_ops.CUSTOM_DVE_SPECS[name] = spec
    dve_ops._SUB_OPCODE_FOR_NAME[name] = row
    return op


EXP_CORR = _make_exp_corr()


def _build(n=N):
    nc = bacc.Bacc("TRN2", target_bir_lowering=False, debug=False)

    xT = nc.declare_dram_parameter("xT", [D, n], BF16, isOutput=False)
    wqT = nc.declare_dram_parameter("wqT", [D, DG], BF16, isOutput=False)
    wkT = nc.declare_dram_parameter("wkT", [D, DG], BF16, isOutput=False)
    wvT = nc.declare_dram_parameter("wvT", [D, DG], BF16, isOutput=False)
    wpT2 = nc.declare_dram_parameter("wpT2", [128, 2, D], BF16, isOutput=False)
    out = nc.declare_dram_parameter("out", [n, D], BF16, isOutput=True)

    DT = D // 128        # 8 contraction tiles for qkv
    NT = n // 128        # key tiles
    QC = min(1024, n)    # qkv prefix group width
    NQC = n // QC
    NC = n // 512        # attention i-chunks

    xT_r = xT.rearrange("(dt p) n -> dt p n", p=128)

    with tile.TileContext(nc) as tc:
        with (
            tc.tile_pool(name="sb", bufs=1) as sb,
            tc.tile_pool(name="wkp", bufs=1) as wkpool,
            tc.tile_pool(name="ps", bufs=1, space="PSUM") as ps,
        ):
            # ---- persistent SBUF tiles ----
            xt = sb.tile([128, DT, n], BF16, tag="xt")
            wq_t = sb.tile([128, DT, DG], BF16, tag="wq")
            wk_t = sb.tile([128, DT, DG], BF16, tag="wk")
            wv_t = sb.tile([128, DT, DG], BF16, tag="wv")
            wp_t = sb.tile([128, 2, D], BF16, tag="wp")
            qt = sb.tile([128, 2, n], BF16, tag="qt")
            kt = sb.tile([128, 2, n], BF16, tag="kt")
            # V stationary, heads at stride 65 (64 V dims + ones col);
            # head h's AV weight window is [65h : 65h+128] (the tail 63
            # cols are the next head's V / zero pad): a full 128-col
            # weight enables the FWL fast path, out rows 65:127 are
            # garbage and never read, the denominator stays at row 64.
            vaug = sb.tile([128, NT, 325], BF16, tag="vaug")
            otn = sb.tile([128, 2, n], BF16, tag="otn")

            # ---- DMA emission: first-needed first, batched per tensor ----
            wqT_p = wqT.rearrange("(dt p) m -> p dt m", p=128)
            wkT_p = wkT.rearrange("(dt p) m -> p dt m", p=128)
            wvT_p = wvT.rearrange("(dt p) m -> p dt m", p=128)
            xT_p = xT.rearrange("(dt p) n -> p dt n", p=128)
            # load the exp table-set (~2.7us) while the DMAs stream
            scr = wkpool.tile([1, 8], FP32, tag="scr", bufs=1, name="scr")
            nc.vector.memset(scr[:, :], 0.0)
            nc.scalar.activation(scr[:, :], scr[:, :], EXP)
            onesb = wkpool.tile([1, 64], BF16, tag="onesb", bufs=1,
                                name="onesb")
            nc.vector.memset(onesb[:, :], 1.0)
            c3t = wkpool.tile([128, 1], FP32, tag="c3t", bufs=1, name="c3t")
            nc.vector.memset(c3t[:, :], EXP_C3)
            # wk and the first x chunk in half-batches: the pre-head K
            # matmuls for dt 0-3 start as soon as the first halves land
            nc.sync.dma_start(wk_t[:, 0:4, :], wkT_p[:, 0:4, :])
            nc.sync.dma_start(xt[:, 0:4, 0:512], xT_p[:, 0:4, 0:512])
            nc.sync.dma_start(wk_t[:, 4:8, :], wkT_p[:, 4:8, :])
            nc.sync.dma_start(xt[:, 4:8, 0:512], xT_p[:, 4:8, 0:512])
            nc.sync.dma_start(wq_t[:, 0:4, :], wqT_p[:, 0:4, :])
            nc.sync.dma_start(wq_t[:, 4:8, :], wqT_p[:, 4:8, :])
            nc.sync.dma_start(wv_t[:, 0:4, :], wvT_p[:, 0:4, :])
            nc.sync.dma_start(xt[:, :, 512:1024], xT_p[:, :, 512:1024])
            nc.sync.dma_start(wv_t[:, 4:8, :], wvT_p[:, 4:8, :])
            nc.sync.dma_start(xt[:, :, 1024:2048], xT_p[:, :, 1024:2048])
            nc.sync.dma_start(wp_t[:, :, :], wpT2[:, :, :])
            nc.sync.dma_start(xt[:, :, 2048:3072], xT_p[:, :, 2048:3072])
            nc.sync.dma_start(xt[:, :, 3072:4096], xT_p[:, :, 3072:4096])
            nc.vector.memset(vaug[:, :, 260:325], 0.0)
            for j in range(NT):
                vj = vaug[:, j, 0:260].rearrange("p (h s) -> p h s", s=65)
                nc.vector.memset(vj[:, :, 64], 1.0)

            # ---- background micro-step machinery ----
            bg = deque()

            def drain(k):
                done = 0
                while bg and done < k:
                    try:
                        next(bg[0])
                        done += 1
                    except StopIteration:
                        bg.popleft()

            # ---- QKV building blocks ----
            def qk_group_bg(w_t, dst, m, c5):
                # [128, 512] background group on the aux tag
                kp = ps.tile([128, 512], FP32, tag="aux", bufs=2, name="qkb")
                for dt_i in range(DT):
                    nc.tensor.matmul(
                        kp[:, :],
                        w_t[:, dt_i, m * 128:(m + 1) * 128],
                        xt[:, dt_i, c5 * 512:(c5 + 1) * 512],
                        start=(dt_i == 0), stop=(dt_i == DT - 1),
                    )
                    yield
                nc.vector.tensor_copy(dst[:, m, c5 * 512:(c5 + 1) * 512],
                                      kp[:, :])
                yield

            def v_gen(j):
                vp = ps.tile([128, DG], FP32, tag="aux", bufs=2, name="vp")
                for dt_i in range(DT):
                    nc.tensor.matmul(
                        vp[:, :],
                        xt[:, dt_i, j * 128:(j + 1) * 128],
                        wv_t[:, dt_i, :],
                        start=(dt_i == 0), stop=(dt_i == DT - 1),
                    )
                    yield
                nc.vector.tensor_copy(
                    vaug[:, j, 0:260].rearrange(
                        "p (h s) -> p h s", s=65)[:, :, 0:64],
                    vp[:, :].rearrange("p (h d) -> p h d", d=64))
                yield

            # ---- normalize + projection generators ----
            def norm_rest(osb, zrow, hh, c):
                pt, odd = hh // 2, hh % 2
                rz = wkpool.tile([1, 512], FP32, tag="rz", bufs=2, name="rz")
                nc.vector.reciprocal_approx_fast(rz[:, :], zrow[:, :])
                yield
                rzs = wkpool.tile([64, 512], FP32, tag="rzs", bufs=4,
                                  name="rzs")
                nc.gpsimd.partition_broadcast(rzs[:, :], rz[:, :])
                yield
                cs = slice(c * 512, (c + 1) * 512)
                if odd == 0:
                    nc.vector.tensor_tensor(otn[0:64, pt, cs],
                                            osb[0:64, :], rzs[:, :], MULT)
                    yield
                else:
                    ohst = wkpool.tile([64, 512], BF16, tag="ohst", bufs=4,
                                       name="ohst")
                    nc.vector.tensor_tensor(ohst[:, :], osb[0:64, :],
                                            rzs[:, :], MULT)
                    yield
                    nc.sync.dma_start(otn[64:128, pt, cs], ohst[:, :])
                    yield

            def norm_last(osb, zrow, hh, c):
                # tail variant: broadcast 1/z with a K=1 PE matmul into a
                # freed ot PSUM bank instead of the serial gpsimd path.
                pt, odd = hh // 2, hh % 2
                rz = wkpool.tile([1, 512], FP32, tag="rz", bufs=2,
                                 name="rz")
                nc.vector.reciprocal_approx_fast(rz[:, :], zrow[:, :])
                rzb = wkpool.tile([1, 512], BF16, tag="rzb", bufs=2,
                                  name="rzb")
                nc.vector.tensor_copy(rzb[:, :], rz[:, :])
                yield
                rzp = ps.tile([128, 512], FP32, tag="ot", bufs=2, name="rzp")
                nc.tensor.matmul(rzp[0:64, :], onesb[0:1, :], rzb[0:1, :],
                                 start=True, stop=True)
                yield
                cs = slice(c * 512, (c + 1) * 512)
                if odd == 0:
                    nc.vector.tensor_tensor(otn[0:64, pt, cs],
                                            osb[0:64, :], rzp[0:64, :], MULT)
                    yield
                else:
                    ohst = wkpool.tile([64, 512], BF16, tag="ohst", bufs=4,
                                       name="ohst")
                    nc.vector.tensor_tensor(ohst[:, :], osb[0:64, :],
                                            rzp[0:64, :], MULT)
                    yield
                    nc.sync.dma_start(otn[64:128, pt, cs], ohst[:, :])
                    yield

            def proj_gen(c):
                for isub in range(4):
                    ib = c * 512 + isub * 128
                    for e in range(2):
                        pj = ps.tile([128, 512], FP32, tag="aux", bufs=2,
                                     name="pj")
                        for pt in range(2):
                            nc.tensor.matmul(
                                pj[:, :],
                                otn[:, pt, ib:ib + 128],
                                wp_t[:, pt, e * 512:(e + 1) * 512],
                                start=(pt == 0), stop=(pt == 1))
                            yield
                        ob = wkpool.tile([128, 512], BF16, tag="ob", bufs=3,
                                         name="ob")
                        nc.vector.tensor_copy(ob[:, :], pj[:, :])
                        nc.sync.dma_start(
                            out[ib:ib + 128, e * 512:(e + 1) * 512],
                            ob[:, :])
                        yield

            # ---- minimal pre-head: K/Q for pair 0, first 512 cols ----
            for _ in qk_group_bg(wk_t, kt, 0, 0):
                pass
            for _ in qk_group_bg(wq_t, qt, 0, 0):
                pass

            # K/Q production queue consumed inside the c0 j-loops.
            # Order = first-needed-first:
            #  c0p0 j>=4 needs K(m0) cols 512:4096; c0p1 needs Q(m1) col
            #  chunk 0 and K(m1) everything; chunk c1 needs Q(*) chunk 1.
            prodq = deque()
            for c5 in range(1, NC):
                prodq.append(qk_group_bg(wk_t, kt, 0, c5))
            prodq.append(qk_group_bg(wq_t, qt, 1, 0))
            prodq.append(qk_group_bg(wk_t, kt, 1, 0))
            for c5 in range(1, NC):
                prodq.append(qk_group_bg(wk_t, kt, 1, c5))

            def drive(q, k):
                done = 0
                while q and done < k:
                    try:
                        next(q[0])
                        done += 1
                    except StopIteration:
                        q.popleft()

            vq = deque(v_gen(j) for j in range(NT))

            # remaining Q as background (512-wide groups); chunk-1 Q
            # first (needed at c1, drained during c0p1)
            bg.append(qk_group_bg(wq_t, qt, 0, 1))
            bg.append(qk_group_bg(wq_t, qt, 1, 1))
            for c5 in range(QC // 512, NC):
                bg.append(qk_group_bg(wq_t, qt, 0, c5))
                bg.append(qk_group_bg(wq_t, qt, 1, c5))

            # ---- attention backbone ----
            for c in range(NC):
                for p in range(2):
                    first_pass = (c == 0 and p == 0)
                    he, ho = 2 * p, 2 * p + 1
                    ot_e = ps.tile([128, 512], FP32, tag="ot", bufs=2,
                                   name="ot_e")
                    ot_o = ps.tile([128, 512], FP32, tag="ot", bufs=2,
                                   name="ot_o")
                    def qk_pair(j):
                        st = ps.tile([128, 1024], FP32, tag="st", bufs=2,
                                     name="st")
                        nc.tensor.matmul(
                            st[:, 0:512],
                            kt[0:64, p, j * 128:(j + 1) * 128],
                            qt[0:64, p, c * 512:(c + 1) * 512],
                            start=True, stop=True)
                        nc.tensor.matmul(
                            st[:, 512:1024],
                            kt[64:128, p, j * 128:(j + 1) * 128],
                            qt[64:128, p, c * 512:(c + 1) * 512],
                            start=True, stop=True)
                        return st

                    avq = deque()
                    nav = [0]

                    def issue_av(pj_, pet, stop):
                        nc.tensor.matmul(
                            ot_e[0:128, :],
                            vaug[:, pj_, 65 * he:65 * he + 128],
                            pet[:, 0:512],
                            start=(nav[0] == 0), stop=stop)
                        nc.tensor.matmul(
                            ot_o[0:128, :],
                            vaug[:, pj_, 65 * ho:65 * ho + 128],
                            pet[:, 512:1024],
                            start=(nav[0] == 0), stop=stop)
                        nav[0] += 1

                    for j in range(NT):
                        st = qk_pair(j)
                        dve_tile = c > 0 and j % 4 == 2
                        if dve_tile:
                            et = sb.tile([128, 1024], BF16, tag="etd",
                                         bufs=3, name="etd")
                        else:
                            et = sb.tile([128, 1024], BF16, tag="et",
                                         bufs=5, name="et")
                        if dve_tile:
                            u16 = sb.tile([128, 1024], I16, tag="u16",
                                          bufs=2, name="u16")
                            gb = sb.tile([128, 1024], BF16, tag="gb",
                                         bufs=2, name="gb")
                            nc.vector.tensor_scalar(
                                u16[:, :], st[:, :], EXP_A, EXP_B, MULT, ADD)
                            nc.vector._custom_dve(
                                EXP_CORR, out=gb[:, :], in0=u16[:, :],
                                in1=c3t[:, :], s0=EXP_C0, s1=EXP_C1,
                                imm2=EXP_C2)
                            nc.vector.tensor_tensor(
                                et[:, :], gb[:, :],
                                u16[:, :].bitcast(BF16), MULT)
                        else:
                            nc.scalar.activation(et[:, :], st[:, :], EXP,
                                                 scale=SCALE)
                        avq.append((j + (4 if dve_tile else 1), j, et))
                        while avq and avq[0][0] <= j:
                            _, pj_, pet = avq.popleft()
                            issue_av(pj_, pet, stop=False)
                        if first_pass:
                            # produce V[j] (needed by AV next iter) and
                            # push K/Q production along behind the QK/AV.
                            drive(vq, 9)
                            drive(prodq, 4)
                        elif c == 0 and p == 1:
                            drive(prodq, 3)
                            drain(1)
                        else:
                            drain(2 if c >= NC - 2 else 1)
                    while avq:
                        _, pj_, pet = avq.popleft()
                        issue_av(pj_, pet, stop=(not avq))
                    # free the PSUM accumulators with one copy each;
                    # the rest of the normalize chain runs in background
                    last_norms = []
                    pairs = ((he, ot_e), (ho, ot_o))
                    if c == NC - 1 and p == 1:
                        pairs = ((ho, ot_o), (he, ot_e))
                    for hh, ot_h in pairs:
                        osb = wkpool.tile([64, 512], BF16, tag="osb",
                                          bufs=6, name="osb")
                        nc.vector.tensor_copy(osb[:, :], ot_h[0:64, :])
                        zrow = wkpool.tile([1, 512], FP32, tag="zrow",
                                           bufs=4, name="zrow")
                        nc.vector.tensor_copy(zrow[:, :], ot_h[64:65, :])
                        if c == NC - 1 and p == 1:
                            last_norms.append(
                                norm_last(osb, zrow, hh, c))
                        else:
                            bg.append(norm_rest(osb, zrow, hh, c))
                    while last_norms:
                        for ng in list(last_norms):
                            try:
                                next(ng)
                            except StopIteration:
                                last_norms.remove(ng)
                pg = proj_gen(c)
                if c == NC - 1:
                    while bg:
                        drain(64)
                    for _ in pg:
                        pass
                else:
                    bg.append(pg)

            while prodq:
                drive(prodq, 64)
            while vq:
                drive(vq, 64)
            while bg:
                drain(64)

    nc.compile()
    return nc


_CACHED = {}


def _get_nc(n=N):
    if n not in _CACHED:
        _CACHED[n] = _build(n)
    return _CACHED[n]


def _make_in_maps(x, w_qkv, w_proj):
    import ml_dtypes
    bf16 = ml_dtypes.bfloat16
    in_maps = []
    for c in range(8):
        b, g = divmod(c, 4)
        s = slice(g * DG, (g + 1) * DG)
        wp = w_proj[:, s]  # [D(e), 256]
        in_maps.append({
            "xT": np.ascontiguousarray(x[b].T).astype(bf16),
            "wqT": np.ascontiguousarray(w_qkv[0 * D:1 * D][s, :].T).astype(bf16),
            "wkT": np.ascontiguousarray(w_qkv[1 * D:2 * D][s, :].T).astype(bf16),
            "wvT": np.ascontiguousarray(w_qkv[2 * D:3 * D][s, :].T).astype(bf16),
            "wpT2": np.ascontiguousarray(
                wp.T.reshape(2, 128, D).transpose(1, 0, 2)).astype(bf16),
        })
    return in_maps


def kernel(x, w_qkv, w_proj, b_proj):
    from concourse.bass_utils import run_bass_kernel_spmd

    nc = _get_nc(N)
    in_maps = _make_in_maps(np.asarray(x), np.asarray(w_qkv),
                            np.asarray(w_proj))
    res = run_bass_kernel_spmd(nc, in_maps, core_ids=list(range(8)))
    outs = [r["out"].astype(np.float32) for r in res.results]
    full = np.stack([outs[0] + outs[1] + outs[2] + outs[3],
                     outs[4] + outs[5] + outs[6] + outs[7]])
    full += np.asarray(b_proj, dtype=np.float32)[None, None, :]
    return full.astype(np.float32)

